# revision 4
# baseline (speedup 1.0000x reference)
"""HardNegTripletMarginLoss on 8 Trainium2 NeuronCores (Bass/Tile).

Strategy (anchors row-sharded across 8 cores, embeddings replicated):
  - Host: normalize rows (as reference), stable-sort rows by label, and give
    each core a column-ROTATED copy of Xn^T so the core's own anchor block
    sits at local columns [0, 1024). With sorted labels every anchor's
    same-label columns then fall inside 2 statically-known 512-wide column
    tiles per anchor block -- identical tile indices on every core, so one
    SPMD program serves all 8 cores.
  - Device: PSUM tile = -2*G (f32r matmul, full PE rate) and, on the two
    "masked" tiles per block, += 16*same via a one-hot (K=64) matmul.
    min over a row of that tile = hardest-negative distance^2 (same-label
    entries are pushed to >= +14, never the min); max over the masked tiles
    = 16 + hardest-positive (-2G) (the diagonal has the largest G of its
    group so it is never selected unless the group is a singleton, matching
    the reference's invalid->0 semantics).
  - d2 = s_i + s_j - 2G with s_j ~= 1 (rows are unit-normalized; |s_j-1| is
    a few 1e-7, far below tolerance). s_i is applied exactly via a
    per-partition bias when taking sqrt.
  - Host: gather per-anchor losses, loss = sum(per)/count(per>0).

This walrus build rejects instructions carrying >1 sync wait, so
Bass.to_json_bytes is wrapped to split multi-wait instructions into
single-wait Drain carriers on the same engine.
"""

import json
import os
import sys
import types
import ctypes

for _p in ("/opt/trn_rl_repo", "/root/.axon_site/_ro/trn_rl_repo"):
    if os.path.isdir(_p) and _p not in sys.path:
        sys.path.append(_p)

import numpy as np
import concourse.bass as bass
import concourse.tile as tile
from concourse import mybir
from concourse.bass_utils import run_bass_kernel_spmd
from contextlib import ExitStack

P = 128
N = 8192
D = 128
NCORES = 8
M = N // NCORES            # anchors per core
NBLK = M // P              # anchor blocks per core
TW = 512                   # matmul moving-dim tile
CW = 2048                  # PSUM chunk (4 banks) / ACT copy granularity
NCH = N // CW
TPC = CW // TW
BIG = 16.0
MARGIN = 0.05
DIRECT_CHUNK = 2           # this chunk is min-reduced straight from PSUM (DVE 1x)
F32 = mybir.dt.float32
F32R = mybir.dt.float32r

# masked (diagonal-window) global col-tile indices per anchor block l
MASKED_TILES = {0: (15, 0), 1: (15, 0), 2: (0, 1), 3: (0, 1),
                4: (0, 1), 5: (0, 1), 6: (1, 2), 7: (1, 2)}
# column slab layout of the one-hot cols input: tile -> slab slot
OH_SLOT = {15: 0, 0: 1, 1: 2, 2: 3}

LAST_RESULTS = None        # BassKernelResults of the most recent run (for test.py)


def _install_wait_split_patch():
    if getattr(bass.Bass, "_wait_split_patched", False):
        return
    orig = bass.Bass.to_json_bytes

    def patched(self):
        raw = orig(self)
        d = json.loads(raw)
        changed = False
        for fn in d.get("functions", []):
            for blk in fn.get("blocks", []):
                out, k = [], 0
                for ins in blk.get("instructions", []):
                    si = ins.get("sync_info") or {}
                    waits = si.get("on_wait") or []
                    if len(waits) > 1:
                        changed = True
                        for w in waits[:-1]:
                            k += 1
                            out.append({
                                "name": f"{ins['name']}-sw{k}",
                                "opcode": "Drain",
                                "engine": ins["engine"],
                                "ins": [],
                                "outs": [],
                                "is_reset_sema": False,
                                "debug": ins.get("debug", 0),
                                "sync_info": {"on_wait": [w], "on_update": []},
                            })
                        si["on_wait"] = [waits[-1]]
                    out.append(ins)
                blk["instructions"] = out
        return json.dumps(d).encode() if changed else raw

    bass.Bass.to_json_bytes = patched
    bass.Bass._wait_split_patched = True


def _ensure_ntff_hook():
    """Best-effort: restore the axon NTFF profile hook this image dropped."""
    if "antenv.axon_hooks" in sys.modules:
        return
    try:
        lib = ctypes.CDLL("/opt/axon/libaxon_pjrt.so")
        if not hasattr(lib, "axon_start_nrt_profile"):
            return
        from trn_agent_boot.trn_boot import _ntff_profile_via_ctypes
        hook = _ntff_profile_via_ctypes("/opt/axon/libaxon_pjrt.so")
        mod = types.ModuleType("antenv.axon_hooks")
        mod._hook = hook
        mod.get_axon_ntff_profile_hook = lambda: mod._hook
        mod.set_axon_ntff_profile_hook = lambda h: setattr(mod, "_hook", h)
        sys.modules["antenv.axon_hooks"] = mod
        import antenv
        antenv.axon_hooks = mod
    except Exception:
        pass


def _build_nc():
    nc = bass.Bass("TRN2", target_bir_lowering=False, debug=False)
    xt_d = nc.dram_tensor("xt", [P, N], F32R, kind="ExternalInput")
    xa_d = nc.dram_tensor("xm2a", [P, M], F32R, kind="ExternalInput")
    ohc_d = nc.dram_tensor("ohc", [64, 4 * TW], F32R, kind="ExternalInput")
    oha_d = nc.dram_tensor("oha", [64, M], F32R, kind="ExternalInput")
    bmn_d = nc.dram_tensor("bmin", [P, NBLK], F32, kind="ExternalInput")
    bmx_d = nc.dram_tensor("bmax", [P, NBLK], F32, kind="ExternalInput")
    out_d = nc.dram_tensor("per_out", [P, NBLK], F32, kind="ExternalOutput")

    # chunk processing order per block: sbuf-copied chunks first (0, 3 hold
    # every masked tile), then the PSUM-resident TTR chunks (1, 2).
    SB_CHUNKS = (0, 3)
    PS_CHUNKS = (1, 2)
    # masked tile -> (chunk, col-slice within chunk)
    def _tile_slice(tg):
        return tg // TPC, slice((tg % TPC) * TW, (tg % TPC + 1) * TW)

    with tile.TileContext(nc) as tc, ExitStack() as ctx:
        xpool = ctx.enter_context(tc.tile_pool(name="xt", bufs=NCH))
        inpool = ctx.enter_context(tc.tile_pool(name="ins", bufs=1))
        spool = ctx.enter_context(tc.tile_pool(name="scp", bufs=2))
        ppool = ctx.enter_context(tc.tile_pool(name="psum", bufs=2, space="PSUM"))
        accpool = ctx.enter_context(tc.tile_pool(name="acc", bufs=1))
        fpool = ctx.enter_context(tc.tile_pool(name="fin", bufs=4))
        opool = ctx.enter_context(tc.tile_pool(name="out", bufs=1))

        # preload the sqrt activation table during DMA startup so the final
        # sqrt doesn't pay the ~2.7us table-load serially at the tail
        warm = fpool.tile([P, 1], F32, tag="warm")
        nc.vector.memset(warm[:], 1.0)
        nc.scalar.activation(warm[:], warm[:], mybir.ActivationFunctionType.Sqrt)

        # small inputs first so the first block's matmuls start early
        xa = inpool.tile([P, M], F32R, tag="xa")
        nc.sync.dma_start(xa[:], xa_d.ap()[:, :])
        oha = inpool.tile([64, M], F32R, tag="oha")
        nc.sync.dma_start(oha[:], oha_d.ap()[:, :])
        ohc = inpool.tile([64, 4 * TW], F32R, tag="ohc")
        nc.sync.dma_start(ohc[:], ohc_d.ap()[:, :])
        bmn = inpool.tile([P, NBLK], F32, tag="bmn")
        nc.sync.dma_start(bmn[:], bmn_d.ap()[:, :])
        bmx = inpool.tile([P, NBLK], F32, tag="bmx")
        nc.sync.dma_start(bmx[:], bmx_d.ap()[:, :])
        xch = [None] * NCH
        for ch in (*SB_CHUNKS, *PS_CHUNKS):   # DMA in the order blocks consume
            t = xpool.tile([P, CW], F32R, tag="xch")
            nc.sync.dma_start(t[:], xt_d.ap()[:, ch * CW:(ch + 1) * CW])
            xch[ch] = t

        # per-block accumulators: cols [0:NBLK] = mins, [NBLK:2*NBLK] = maxes
        acc = accpool.tile([P, 2 * NBLK], F32, tag="acc", name="acc")

        def _mm_chunk(ps, l, ch):
            lhsT = xa[:, l * P:(l + 1) * P]
            masked_us = []
            for u in range(TPC):
                tg = ch * TPC + u
                m = tg in MASKED_TILES[l]
                if m:
                    masked_us.append((u, tg))
                nc.tensor.matmul(
                    ps[:, u * TW:(u + 1) * TW],
                    lhsT=lhsT,
                    rhs=xch[ch][:, u * TW:(u + 1) * TW],
                    start=True, stop=not m)
            for u, tg in masked_us:
                sl = OH_SLOT[tg]
                nc.tensor.matmul(
                    ps[:, u * TW:(u + 1) * TW],
                    lhsT=oha[:, l * P:(l + 1) * P],
                    rhs=ohc[:, sl * TW:(sl + 1) * TW],
                    start=False, stop=True)

        for l in range(NBLK):
            sb = {}
            for ch in SB_CHUNKS:
                ps = ppool.tile([P, CW], F32, tag="ps")
                _mm_chunk(ps, l, ch)
                s = spool.tile([P, CW], F32, tag=f"s{ch}")
                nc.scalar.copy(s[:], ps[:])
                sb[ch] = s
            # hardest-negative: chained 2-input running-min scans, each
            # consuming a PSUM chunk + an SBUF chunk at 2 elems/cycle on DVE
            # (vs tensor_reduce's 1); the second scan's last element is the
            # block min over all 4 chunks
            scr = None
            for k, ch in enumerate(PS_CHUNKS):
                ps = ppool.tile([P, CW], F32, tag="ps")
                _mm_chunk(ps, l, ch)
                init = 1e30 if scr is None else scr[:, CW - 1:CW]
                nscr = spool.tile([P, CW], F32, tag=f"scr{k}")
                nc.vector.tensor_tensor_scan(
                    nscr[:], ps[:], sb[SB_CHUNKS[k]][:], initial=init,
                    op0=mybir.AluOpType.min, op1=mybir.AluOpType.min)
                scr = nscr
            nc.vector.tensor_copy(acc[:, l:l + 1], scr[:, CW - 1:CW])
            # hardest-positive: single 2-input running-max scan over the two
            # masked 512-tiles (both in SBUF copies)
            (ca, sa), (cb, sbl) = (_tile_slice(tg) for tg in MASKED_TILES[l])
            mscr = spool.tile([P, TW], F32, tag="mscr")
            nc.vector.tensor_tensor_scan(
                mscr[:], sb[ca][:, sa], sb[cb][:, sbl], initial=-1e30,
                op0=mybir.AluOpType.max, op1=mybir.AluOpType.max)
            nc.vector.tensor_copy(acc[:, NBLK + l:NBLK + l + 1],
                                  mscr[:, TW - 1:TW])

        per_all = opool.tile([P, NBLK], F32, tag="per")
        d2b = fpool.tile([P, 2 * NBLK], F32, tag="d2b")
        nc.vector.tensor_add(d2b[:, 0:NBLK], acc[:, 0:NBLK], bmn[:])
        nc.vector.tensor_add(d2b[:, NBLK:2 * NBLK], acc[:, NBLK:2 * NBLK], bmx[:])
        nc.vector.tensor_scalar(out=d2b[:], in0=d2b[:], scalar1=0.0, scalar2=None,
                                op0=mybir.AluOpType.max)
        dr = fpool.tile([P, 2 * NBLK], F32, tag="dr")
        nc.scalar.activation(dr[:], d2b[:], mybir.ActivationFunctionType.Sqrt)
        df = fpool.tile([P, NBLK], F32, tag="df")
        nc.vector.tensor_sub(df[:], dr[:, NBLK:2 * NBLK], dr[:, 0:NBLK])
        nc.vector.tensor_scalar(out=per_all[:], in0=df[:],
                                scalar1=MARGIN, scalar2=0.0,
                                op0=mybir.AluOpType.add, op1=mybir.AluOpType.max)
        nc.sync.dma_start(out_d.ap()[:, :], per_all[:])
    return nc


def _reference_fallback(embeddings, labels):
    x = embeddings / np.maximum(
        np.sqrt((embeddings * embeddings).sum(1, keepdims=True)), 1e-12)
    sq = (x * x).sum(1)
    d2 = sq[:, None] + sq[None, :] - 2.0 * (x @ x.T)
    dist = np.sqrt(np.maximum(d2, 0.0))
    same = labels[:, None] == labels[None, :]
    eye = np.eye(len(labels), dtype=bool)
    pos, neg = same & ~eye, ~same
    d_ap = np.where(pos, dist, -np.inf).max(1)
    d_an = np.where(neg, dist, np.inf).min(1)
    valid = pos.any(1) & neg.any(1)
    per = np.maximum(d_ap - d_an + MARGIN, 0.0)
    per = np.where(valid, per, 0.0)
    nz = (per > 0).sum()
    return np.float32(per.sum() / max(nz, 1)) if nz > 0 else np.float32(0.0)


def kernel(embeddings: np.ndarray, labels: np.ndarray) -> np.ndarray:
    global LAST_RESULTS
    emb = np.asarray(embeddings, dtype=np.float32)
    lab = np.asarray(labels).reshape(-1)

    counts = np.bincount(lab.astype(np.int64) - lab.min())
    if emb.shape != (N, D) or counts.max() > 256 or len(np.unique(lab)) < 2:
        return np.array(_reference_fallback(emb, lab), dtype=np.float32)

    norms = np.sqrt((emb * emb).sum(1, keepdims=True, dtype=np.float32))
    xn = emb / np.maximum(norms, np.float32(1e-12))
    s = (xn * xn).sum(1, dtype=np.float32)

    perm = np.argsort(lab, kind="stable")
    xs = xn[perm]
    ls = lab[perm]
    ss = s[perm]

    # map labels to dense 0..63 codes for the one-hot
    uniq = np.unique(ls)
    code = np.searchsorted(uniq, ls).astype(np.int64)
    assert len(uniq) <= 64

    _install_wait_split_patch()
    _ensure_ntff_hook()
    nc = _build_nc()

    in_maps = []
    for c in range(NCORES):
        lo = c * M
        rot = np.roll(np.arange(N), -lo)            # local col j -> sorted row
        xt = np.ascontiguousarray(xs[rot].T)        # [128, 8192]
        xm2a = np.ascontiguousarray((-2.0 * xs[lo:lo + M]).T)
        slab = np.concatenate([rot[N - TW:], rot[:3 * TW]])   # local cols 7680:8192 + 0:1536
        ohc = (code[slab][None, :] == np.arange(64)[:, None]).astype(np.float32)
        oha = (BIG * (code[lo:lo + M][None, :] == np.arange(64)[:, None])).astype(np.float32)
        bmin = np.ascontiguousarray((1.0 + ss[lo:lo + M]).reshape(NBLK, P).T.astype(np.float32))
        bmax = (bmin - np.float32(BIG)).astype(np.float32)
        in_maps.append({"xt": xt, "xm2a": xm2a, "ohc": ohc, "oha": oha,
                        "bmin": bmin, "bmax": bmax})

    res = run_bass_kernel_spmd(nc, in_maps, core_ids=list(range(NCORES)))
    LAST_RESULTS = res

    per = np.concatenate(
        [res.results[c]["per_out"].T.reshape(M) for c in range(NCORES)])
    nz = int((per > 0).sum())
    if nz == 0:
        return np.array(0.0, dtype=np.float32)
    return np.array(np.float32(per.sum(dtype=np.float64) / nz), dtype=np.float32)


if __name__ == "__main__":
    # quick native compile smoke (no device run)
    from concourse import bass_utils
    import tempfile
    _install_wait_split_patch()
    nc = _build_nc()
    td = tempfile.mkdtemp(prefix="tripletk_")
    print(bass_utils.compile_bass_kernel(nc, td))



# revision 5
# speedup vs baseline: 1.3351x; 1.3351x over previous
"""HardNegTripletMarginLoss on 8 Trainium2 NeuronCores (Bass/Tile).

Strategy (anchors row-sharded across 8 cores, embeddings replicated):
  - Host: normalize rows (as reference), stable-sort rows by label, and give
    each core a column-ROTATED copy of Xn^T so the core's own anchor block
    sits at local columns [0, 1024). With sorted labels every anchor's
    same-label columns then fall inside 2 statically-known 512-wide column
    tiles per anchor block -- identical tile indices on every core, so one
    SPMD program serves all 8 cores.
  - Device per 128-anchor block (8 col-subchunks of 1024 each):
      * PSUM subchunk = -2*G (f32r matmul, full PE rate); the two "masked"
        512-tiles per block get += 16*same via a one-hot (K=64) matmul.
      * exact half (subchunks 2..5): DVE tensor_reduce min straight from
        PSUM -> per-block exact min m12.
      * soft half (subchunks 0,1,6,7): ACT exp-accumulate softmin
        sum(exp(-T*(v - m12))) straight from PSUM (the m12 bias keeps the
        exponent in range; far values underflow to 0 harmlessly; lifted
        same-label entries vanish automatically).
      * hardest-positive: ACT exp-accumulate softmax over the two masked
        512-tiles, sum(exp(+T*(v - r))) with r a host-computed per-anchor
        sampled positive max (guarantees >= 1 term with exponent ~0).
    This splits the 8.4M-element/core eviction between DVE (~123G elem/s)
    and ACT (~154G elem/s) with no PSUM->SBUF copies at all.
  - Host: ln()/T finals, d2 = s_i + s_j - 2G with s_j ~= 1 folded into
    biases, sqrt, relu, AvgNonZero reduction (tiny, O(N) work).

This walrus build rejects instructions carrying >1 sync wait, so
Bass.to_json_bytes is wrapped to split multi-wait instructions into
single-wait Drain carriers on the same engine.
"""

import json
import os
import sys
import types
import ctypes

for _p in ("/opt/trn_rl_repo", "/root/.axon_site/_ro/trn_rl_repo"):
    if os.path.isdir(_p) and _p not in sys.path:
        sys.path.append(_p)

import numpy as np
import concourse.bass as bass
import concourse.tile as tile
from concourse import mybir
from concourse.bass_utils import run_bass_kernel_spmd
from contextlib import ExitStack

P = 128
N = 8192
D = 128
NCORES = 8
M = N // NCORES            # anchors per core
NBLK = M // P              # anchor blocks per core
TW = 512                   # matmul moving-dim tile
SC = 1024                  # PSUM subchunk width (2 banks)
NSUB = N // SC             # 8 subchunks per block row
EXACT_SUBS = (2, 3, 4, 5)  # DVE exact-min subchunks (cols 2048:6144)
SOFT_SUBS = (0, 1, 6, 7)   # ACT softmin subchunks
BIG = 16.0
MARGIN = 0.05
TSOFT = 64.0               # softmin/softmax temperature (in d^2 units)
F32 = mybir.dt.float32
F32R = mybir.dt.float32r

# masked (diagonal-window) global 512-col tile indices per anchor block l
MASKED_TILES = {0: (15, 0), 1: (15, 0), 2: (0, 1), 3: (0, 1),
                4: (0, 1), 5: (0, 1), 6: (1, 2), 7: (1, 2)}
# column slab layout of the one-hot cols input: tile -> slab slot
OH_SLOT = {15: 0, 0: 1, 1: 2, 2: 3}

# output layout: [P, 56] = m12[8] | ssm[32] | msk[16]
OUT_W = NBLK + 4 * NBLK + 2 * NBLK

LAST_RESULTS = None        # BassKernelResults of the most recent run (for test.py)


def _install_wait_split_patch():
    if getattr(bass.Bass, "_wait_split_patched", False):
        return
    orig = bass.Bass.to_json_bytes

    def patched(self):
        raw = orig(self)
        d = json.loads(raw)
        changed = False
        for fn in d.get("functions", []):
            for blk in fn.get("blocks", []):
                out, k = [], 0
                for ins in blk.get("instructions", []):
                    si = ins.get("sync_info") or {}
                    waits = si.get("on_wait") or []
                    if len(waits) > 1:
                        changed = True
                        for w in waits[:-1]:
                            k += 1
                            out.append({
                                "name": f"{ins['name']}-sw{k}",
                                "opcode": "Drain",
                                "engine": ins["engine"],
                                "ins": [],
                                "outs": [],
                                "is_reset_sema": False,
                                "debug": ins.get("debug", 0),
                                "sync_info": {"on_wait": [w], "on_update": []},
                            })
                        si["on_wait"] = [waits[-1]]
                    out.append(ins)
                blk["instructions"] = out
        return json.dumps(d).encode() if changed else raw

    bass.Bass.to_json_bytes = patched
    bass.Bass._wait_split_patched = True


def _ensure_ntff_hook():
    """Best-effort: restore the axon NTFF profile hook this image dropped."""
    if "antenv.axon_hooks" in sys.modules:
        return
    try:
        lib = ctypes.CDLL("/opt/axon/libaxon_pjrt.so")
        if not hasattr(lib, "axon_start_nrt_profile"):
            return
        from trn_agent_boot.trn_boot import _ntff_profile_via_ctypes
        hook = _ntff_profile_via_ctypes("/opt/axon/libaxon_pjrt.so")
        mod = types.ModuleType("antenv.axon_hooks")
        mod._hook = hook
        mod.get_axon_ntff_profile_hook = lambda: mod._hook
        mod.set_axon_ntff_profile_hook = lambda h: setattr(mod, "_hook", h)
        sys.modules["antenv.axon_hooks"] = mod
        import antenv
        antenv.axon_hooks = mod
    except Exception:
        pass


def _build_nc():
    nc = bass.Bass("TRN2", target_bir_lowering=False, debug=False)
    xt_d = nc.dram_tensor("xt", [P, N], F32R, kind="ExternalInput")
    xa_d = nc.dram_tensor("xm2a", [P, M], F32R, kind="ExternalInput")
    ohc_d = nc.dram_tensor("ohc", [64, 4 * TW], F32R, kind="ExternalInput")
    oha_d = nc.dram_tensor("oha", [64, M], F32R, kind="ExternalInput")
    brf_d = nc.dram_tensor("brf", [P, NBLK], F32, kind="ExternalInput")
    out_d = nc.dram_tensor("per_out", [P, OUT_W], F32, kind="ExternalOutput")

    with tile.TileContext(nc) as tc, ExitStack() as ctx:
        xpool = ctx.enter_context(tc.tile_pool(name="xt", bufs=4))
        inpool = ctx.enter_context(tc.tile_pool(name="ins", bufs=1))
        ppool = ctx.enter_context(tc.tile_pool(name="psum", bufs=4, space="PSUM"))
        scpool = ctx.enter_context(tc.tile_pool(name="scr", bufs=2))
        accpool = ctx.enter_context(tc.tile_pool(name="acc", bufs=1))
        fpool = ctx.enter_context(tc.tile_pool(name="fin", bufs=2))

        # preload the exp activation table during the input DMAs so the first
        # soft subchunk doesn't pay the ~2.7us table load
        warm = fpool.tile([P, 1], F32, tag="warm")
        nc.vector.memset(warm[:], 0.0)
        nc.scalar.activation(warm[:], warm[:], mybir.ActivationFunctionType.Exp)

        # small inputs first so the first block's matmuls start early
        xa = inpool.tile([P, M], F32R, tag="xa")
        nc.sync.dma_start(xa[:], xa_d.ap()[:, :])
        oha = inpool.tile([64, M], F32R, tag="oha")
        nc.sync.dma_start(oha[:], oha_d.ap()[:, :])
        ohc = inpool.tile([64, 4 * TW], F32R, tag="ohc")
        nc.sync.dma_start(ohc[:], ohc_d.ap()[:, :])
        brf = inpool.tile([P, NBLK], F32, tag="brf")
        nc.sync.dma_start(brf[:], brf_d.ap()[:, :])
        # xt in 4 chunk pieces, exact-half columns first (consumed first)
        xch = [None] * 4
        for ch in (1, 2, 0, 3):
            t = xpool.tile([P, 2 * SC], F32R, tag="xch")
            nc.sync.dma_start(t[:], xt_d.ap()[:, ch * 2 * SC:(ch + 1) * 2 * SC])
            xch[ch] = t

        def xcol(tg):
            """rhs slice of global 512-tile tg from the chunked xt tiles."""
            ch, off = tg // 4, (tg % 4) * TW
            return xch[ch][:, off:off + TW]

        acc_e = accpool.tile([P, 4 * NBLK], F32, tag="acc_e", name="acc_e")
        m12 = accpool.tile([P, NBLK], F32, tag="m12", name="m12")
        ssm = accpool.tile([P, 4 * NBLK], F32, tag="ssm", name="ssm")
        msk = accpool.tile([P, 2 * NBLK], F32, tag="msk", name="msk")
        bias_mn = accpool.tile([P, NBLK], F32, tag="bias_mn", name="bias_mn")

        def mm_sub(ps, l, u):
            """matmul subchunk u (global tiles 2u, 2u+1) for block l into ps."""
            lhsT = xa[:, l * P:(l + 1) * P]
            masked = []
            for half in range(2):
                tg = 2 * u + half
                m = tg in MASKED_TILES[l]
                if m:
                    masked.append((half, tg))
                nc.tensor.matmul(
                    ps[:, half * TW:(half + 1) * TW],
                    lhsT=lhsT, rhs=xcol(tg), start=True, stop=not m)
            for half, tg in masked:
                sl = OH_SLOT[tg]
                nc.tensor.matmul(
                    ps[:, half * TW:(half + 1) * TW],
                    lhsT=oha[:, l * P:(l + 1) * P],
                    rhs=ohc[:, sl * TW:(sl + 1) * TW],
                    start=False, stop=True)

        for l in range(NBLK):
            # exact half: DVE min-reduce straight from PSUM
            for k, u in enumerate(EXACT_SUBS):
                ps = ppool.tile([P, SC], F32, tag="ps")
                mm_sub(ps, l, u)
                nc.vector.tensor_reduce(
                    acc_e[:, l * 4 + k:l * 4 + k + 1], ps[:],
                    op=mybir.AluOpType.min, axis=mybir.AxisListType.X)
            nc.vector.tensor_reduce(
                m12[:, l:l + 1], acc_e[:, l * 4:(l + 1) * 4],
                op=mybir.AluOpType.min, axis=mybir.AxisListType.X)
            nc.vector.tensor_scalar(
                out=bias_mn[:, l:l + 1], in0=m12[:, l:l + 1],
                scalar1=TSOFT, scalar2=None, op0=mybir.AluOpType.mult)
            # soft half: ACT exp-accumulate straight from PSUM
            nmsk = 0
            for j, u in enumerate(SOFT_SUBS):
                ps = ppool.tile([P, SC], F32, tag="ps")
                mm_sub(ps, l, u)
                # masked softmax first (needs the original values)
                for half in range(2):
                    tg = 2 * u + half
                    if tg in MASKED_TILES[l]:
                        sm = scpool.tile([P, TW], F32, tag="sm")
                        nc.scalar.activation(
                            sm[:], ps[:, half * TW:(half + 1) * TW],
                            mybir.ActivationFunctionType.Exp,
                            bias=brf[:, l:l + 1], scale=TSOFT,
                            accum_out=msk[:, 2 * l + nmsk:2 * l + nmsk + 1])
                        nmsk += 1
                sb = scpool.tile([P, SC], F32, tag="sb")
                nc.scalar.activation(
                    sb[:], ps[:], mybir.ActivationFunctionType.Exp,
                    bias=bias_mn[:, l:l + 1], scale=-TSOFT,
                    accum_out=ssm[:, l * 4 + j:l * 4 + j + 1])
            assert nmsk == 2, (l, nmsk)

        outt = fpool.tile([P, OUT_W], F32, tag="outt")
        nc.vector.tensor_copy(outt[:, 0:NBLK], m12[:])
        nc.vector.tensor_copy(outt[:, NBLK:5 * NBLK], ssm[:])
        nc.vector.tensor_copy(outt[:, 5 * NBLK:7 * NBLK], msk[:])
        nc.sync.dma_start(out_d.ap()[:, :], outt[:])
    return nc


def _reference_fallback(embeddings, labels):
    x = embeddings / np.maximum(
        np.sqrt((embeddings * embeddings).sum(1, keepdims=True)), 1e-12)
    sq = (x * x).sum(1)
    d2 = sq[:, None] + sq[None, :] - 2.0 * (x @ x.T)
    dist = np.sqrt(np.maximum(d2, 0.0))
    same = labels[:, None] == labels[None, :]
    eye = np.eye(len(labels), dtype=bool)
    pos, neg = same & ~eye, ~same
    d_ap = np.where(pos, dist, -np.inf).max(1)
    d_an = np.where(neg, dist, np.inf).min(1)
    valid = pos.any(1) & neg.any(1)
    per = np.maximum(d_ap - d_an + MARGIN, 0.0)
    per = np.where(valid, per, 0.0)
    nz = (per > 0).sum()
    return np.float32(per.sum() / max(nz, 1)) if nz > 0 else np.float32(0.0)


def kernel(embeddings: np.ndarray, labels: np.ndarray) -> np.ndarray:
    global LAST_RESULTS
    emb = np.asarray(embeddings, dtype=np.float32)
    lab = np.asarray(labels).reshape(-1)

    counts = np.bincount(lab.astype(np.int64) - lab.min())
    if emb.shape != (N, D) or counts.max() > 256 or len(np.unique(lab)) < 2:
        return np.array(_reference_fallback(emb, lab), dtype=np.float32)

    norms = np.sqrt((emb * emb).sum(1, keepdims=True, dtype=np.float32))
    xn = emb / np.maximum(norms, np.float32(1e-12))
    s = (xn * xn).sum(1, dtype=np.float32)

    perm = np.argsort(lab, kind="stable")
    xs = xn[perm]
    ls = lab[perm]
    ss = s[perm]

    # map labels to dense 0..63 codes for the one-hot
    uniq = np.unique(ls)
    code = np.searchsorted(uniq, ls).astype(np.int64)
    assert len(uniq) <= 64

    # per-anchor sampled positive max (lifted units): softmax reference that
    # guarantees at least one exponent ~0 term and bounds the rest
    rng = np.random.default_rng(7)
    r_ref = np.empty(N, dtype=np.float32)
    for g in range(len(uniq)):
        idx = np.where(code == g)[0]
        samp = rng.choice(idx, size=min(16, len(idx)), replace=False)
        gm = xs[idx] @ xs[samp].T
        r_ref[idx] = (-2.0 * gm).max(1) + np.float32(BIG)

    _install_wait_split_patch()
    _ensure_ntff_hook()
    nc = _build_nc()

    in_maps = []
    for c in range(NCORES):
        lo = c * M
        rot = np.roll(np.arange(N), -lo)            # local col j -> sorted row
        xt = np.ascontiguousarray(xs[rot].T)        # [128, 8192]
        xm2a = np.ascontiguousarray((-2.0 * xs[lo:lo + M]).T)
        slab = np.concatenate([rot[N - TW:], rot[:3 * TW]])   # local cols 7680:8192 + 0:1536
        ohc = (code[slab][None, :] == np.arange(64)[:, None]).astype(np.float32)
        oha = (BIG * (code[lo:lo + M][None, :] == np.arange(64)[:, None])).astype(np.float32)
        brf = np.ascontiguousarray(
            (-TSOFT * r_ref[lo:lo + M]).reshape(NBLK, P).T.astype(np.float32))
        in_maps.append({"xt": xt, "xm2a": xm2a, "ohc": ohc, "oha": oha,
                        "brf": brf})

    res = run_bass_kernel_spmd(nc, in_maps, core_ids=list(range(NCORES)))
    LAST_RESULTS = res

    # host finals: per sorted anchor, combine exact min, softmin, softmax
    d_ap_all = np.empty(N, dtype=np.float64)
    d_an_all = np.empty(N, dtype=np.float64)
    for c in range(NCORES):
        o = res.results[c]["per_out"]               # [P, 56]
        lo = c * M
        m12 = o[:, 0:NBLK].T.reshape(M)             # [M] block-major
        ssum = (o[:, NBLK:5 * NBLK].astype(np.float64)
                .reshape(P, NBLK, 4).sum(2).T.reshape(M))
        msum = (o[:, 5 * NBLK:7 * NBLK].astype(np.float64)
                .reshape(P, NBLK, 2).sum(2).T.reshape(M))
        mn_soft = m12 - np.log(np.maximum(ssum, 1e-30)) / TSOFT
        mn = np.minimum(m12, mn_soft)
        mx = np.log(np.maximum(msum, 1e-30)) / TSOFT + r_ref[lo:lo + M]
        s_i = ss[lo:lo + M]
        d_an_all[lo:lo + M] = np.sqrt(np.maximum(s_i + 1.0 + mn, 0.0))
        d_ap_all[lo:lo + M] = np.sqrt(np.maximum(s_i + 1.0 + mx - BIG, 0.0))

    per = np.maximum(d_ap_all - d_an_all + MARGIN, 0.0)
    nz = int((per > 0).sum())
    if nz == 0:
        return np.array(0.0, dtype=np.float32)
    return np.array(np.float32(per.sum() / nz), dtype=np.float32)


if __name__ == "__main__":
    # quick native compile smoke (no device run)
    from concourse import bass_utils
    import tempfile
    _install_wait_split_patch()
    nc = _build_nc()
    td = tempfile.mkdtemp(prefix="tripletk_")
    print(bass_utils.compile_bass_kernel(nc, td))


# revision 7
# speedup vs baseline: 1.7969x; 1.3459x over previous
"""HardNegTripletMarginLoss on 8 Trainium2 NeuronCores (Bass/Tile).

Strategy (anchors row-sharded across 8 cores, embeddings replicated):
  - Host: normalize rows (as reference), stable-sort rows by label, and give
    each core a column-ROTATED copy of Xn^T (quantized to fp8-e4m3: same PE
    rate as f32r at K=128 but 4x less DMA/SBUF) so the core's own anchor
    block sits at local columns [0, 1024). With sorted labels every anchor's
    same-label columns fall inside 2 statically-known 512-wide column tiles
    per anchor block -- identical tile indices on every core, so one SPMD
    program serves all 8 cores.
  - Device per 128-anchor block: 16 column-tiles of 512 packed into 8 PSUM
    containers of [128,1024] (the 2 masked/same-label tiles share container
    0, which gets += 16*same via a one-hot matmul):
      * exact containers: DVE tensor_reduce min straight from PSUM.
      * soft containers: ACT exp-accumulate sum(exp(-T*(v - r_mn))) straight
        from PSUM (softmin; r_mn is a host-sampled per-anchor reference that
        keeps the exponent in range; lifted same-label entries vanish).
      * hardest-positive: ACT exp-accumulate sum(exp(+T*(v - r_mx))) over
        container 0 (softmax; non-lifted entries vanish; r_mx is a
        host-sampled positive max so >= 1 term has exponent ~0).
    This splits the 8.4M-element/core eviction between DVE (~123G elem/s)
    and ACT (~154G elem/s) with no PSUM->SBUF copies and no cross-engine
    data dependencies; exact/soft issue is interleaved so both engines and
    the PE run concurrently through the 4-deep PSUM container ring.
  - Host: ln()/T finals, d2 = s_i + s_j - 2G with s_j ~= 1, sqrt, relu,
    AvgNonZero reduction, plus exact re-repair of any overflowed anchors
    (tiny, O(N) work).

This walrus build rejects instructions carrying >1 sync wait, so
Bass.to_json_bytes is wrapped to split multi-wait instructions into
single-wait Drain carriers on the same engine.
"""

import json
import os
import sys
import types
import ctypes

for _p in ("/opt/trn_rl_repo", "/root/.axon_site/_ro/trn_rl_repo"):
    if os.path.isdir(_p) and _p not in sys.path:
        sys.path.append(_p)

import numpy as np
import ml_dtypes
import concourse.bass as bass
import concourse.tile as tile
from concourse import mybir
from concourse.bass_utils import run_bass_kernel_spmd
from contextlib import ExitStack

P = 128
N = 8192
D = 128
NCORES = 8
M = N // NCORES            # anchors per core
NBLK = M // P              # anchor blocks per core
TW = 512                   # matmul moving-dim tile / half-container
SC = 1024                  # PSUM container width (2 banks)
BIG = 16.0
MARGIN = 0.05
TSOFT = 64.0               # softmin/softmax temperature (in d^2 units)
F32 = mybir.dt.float32
FP8 = mybir.dt.float8e4
E4M3 = ml_dtypes.float8_e4m3fn

# masked (diagonal-window) global 512-col tile indices per anchor block l
MASKED_TILES = {0: (15, 0), 1: (15, 0), 2: (0, 1), 3: (0, 1),
                4: (0, 1), 5: (0, 1), 6: (1, 2), 7: (1, 2)}
# column slab layout of the one-hot cols input: tile -> slab slot
OH_SLOT = {15: 0, 0: 1, 1: 2, 2: 3}

# number of exact (DVE) containers per block; the rest (8-e) are soft (ACT)
BLOCK_E = (5, 5, 5, 5, 5, 5, 4, 4)


def _block_plan(l):
    """containers[i] = (tile_a, tile_b); container 0 holds the masked pair.
    Returns (containers, exact_idx, soft_idx, issue_order)."""
    mt = sorted(MASKED_TILES[l])
    rest = [t for t in range(16) if t not in mt]
    containers = [tuple(mt)] + [(rest[2 * i], rest[2 * i + 1]) for i in range(7)]
    e = BLOCK_E[l]
    soft = list(range(1, 8 - e + 1))        # c1..c(8-e)
    exact = [0] + list(range(8 - e + 1, 8))  # c0 + tail
    # interleave exact/soft for engine overlap
    order = []
    a, b = list(exact), list(soft)
    while a or b:
        if a:
            order.append(a.pop(0))
        if b:
            order.append(b.pop(0))
    return containers, exact, soft, order


# output layout: per block 5 exact-min cols | 4 soft-sum cols | 1 msk col
OUT_W = NBLK * 5 + NBLK * 4 + NBLK

LAST_RESULTS = None        # BassKernelResults of the most recent run (for test.py)


def _install_wait_split_patch():
    if getattr(bass.Bass, "_wait_split_patched", False):
        return
    orig = bass.Bass.to_json_bytes

    def patched(self):
        raw = orig(self)
        d = json.loads(raw)
        changed = False
        for fn in d.get("functions", []):
            for blk in fn.get("blocks", []):
                out, k = [], 0
                for ins in blk.get("instructions", []):
                    si = ins.get("sync_info") or {}
                    waits = si.get("on_wait") or []
                    if len(waits) > 1:
                        changed = True
                        for w in waits[:-1]:
                            k += 1
                            out.append({
                                "name": f"{ins['name']}-sw{k}",
                                "opcode": "Drain",
                                "engine": ins["engine"],
                                "ins": [],
                                "outs": [],
                                "is_reset_sema": False,
                                "debug": ins.get("debug", 0),
                                "sync_info": {"on_wait": [w], "on_update": []},
                            })
                        si["on_wait"] = [waits[-1]]
                    out.append(ins)
                blk["instructions"] = out
        return json.dumps(d).encode() if changed else raw

    bass.Bass.to_json_bytes = patched
    bass.Bass._wait_split_patched = True


def _ensure_ntff_hook():
    """Best-effort: restore the axon NTFF profile hook this image dropped."""
    if "antenv.axon_hooks" in sys.modules:
        return
    try:
        lib = ctypes.CDLL("/opt/axon/libaxon_pjrt.so")
        if not hasattr(lib, "axon_start_nrt_profile"):
            return
        from trn_agent_boot.trn_boot import _ntff_profile_via_ctypes
        hook = _ntff_profile_via_ctypes("/opt/axon/libaxon_pjrt.so")
        mod = types.ModuleType("antenv.axon_hooks")
        mod._hook = hook
        mod.get_axon_ntff_profile_hook = lambda: mod._hook
        mod.set_axon_ntff_profile_hook = lambda h: setattr(mod, "_hook", h)
        sys.modules["antenv.axon_hooks"] = mod
        import antenv
        antenv.axon_hooks = mod
    except Exception:
        pass


def _build_nc():
    nc = bass.Bass("TRN2", target_bir_lowering=False, debug=False)
    xt_d = nc.dram_tensor("xt", [P, N], FP8, kind="ExternalInput")
    xa_d = nc.dram_tensor("xm2a", [P, M], FP8, kind="ExternalInput")
    ohc_d = nc.dram_tensor("ohc", [64, 4 * TW], FP8, kind="ExternalInput")
    oha_d = nc.dram_tensor("oha", [64, M], FP8, kind="ExternalInput")
    bmn_d = nc.dram_tensor("bmn", [P, NBLK], F32, kind="ExternalInput")
    bmx_d = nc.dram_tensor("bmx", [P, NBLK], F32, kind="ExternalInput")
    out_d = nc.dram_tensor("per_out", [P, OUT_W], F32, kind="ExternalOutput")

    with tile.TileContext(nc) as tc, ExitStack() as ctx:
        xpool = ctx.enter_context(tc.tile_pool(name="xt", bufs=4))
        inpool = ctx.enter_context(tc.tile_pool(name="ins", bufs=1))
        ppool = ctx.enter_context(tc.tile_pool(name="psum", bufs=4, space="PSUM"))
        scpool = ctx.enter_context(tc.tile_pool(name="scr", bufs=2))
        accpool = ctx.enter_context(tc.tile_pool(name="acc", bufs=1))
        fpool = ctx.enter_context(tc.tile_pool(name="fin", bufs=2))

        # preload the exp activation table during the input DMAs so the first
        # soft container doesn't pay the ~2.7us table load
        warm = fpool.tile([P, 1], F32, tag="warm")
        nc.vector.memset(warm[:], 0.0)
        nc.scalar.activation(warm[:], warm[:], mybir.ActivationFunctionType.Exp)

        # small inputs first so the first block's matmuls start early
        xa = inpool.tile([P, M], FP8, tag="xa")
        nc.sync.dma_start(xa[:], xa_d.ap()[:, :])
        oha = inpool.tile([64, M], FP8, tag="oha")
        nc.sync.dma_start(oha[:], oha_d.ap()[:, :])
        ohc = inpool.tile([64, 4 * TW], FP8, tag="ohc")
        nc.sync.dma_start(ohc[:], ohc_d.ap()[:, :])
        bmn = inpool.tile([P, NBLK], F32, tag="bmn")
        nc.sync.dma_start(bmn[:], bmn_d.ap()[:, :])
        bmx = inpool.tile([P, NBLK], F32, tag="bmx")
        nc.sync.dma_start(bmx[:], bmx_d.ap()[:, :])
        xch = [None] * 4
        for ch in range(4):
            t = xpool.tile([P, 2 * SC], FP8, tag="xch")
            nc.sync.dma_start(t[:], xt_d.ap()[:, ch * 2 * SC:(ch + 1) * 2 * SC])
            xch[ch] = t

        def xcol(tg):
            ch, off = tg // 4, (tg % 4) * TW
            return xch[ch][:, off:off + TW]

        acc_e = accpool.tile([P, 5 * NBLK], F32, tag="acc_e", name="acc_e")
        ssm = accpool.tile([P, 4 * NBLK], F32, tag="ssm", name="ssm")
        msk = accpool.tile([P, NBLK], F32, tag="msk", name="msk")

        def mm_container(ps, l, pair):
            """matmul the two global tiles of a container into ps halves."""
            lhsT = xa[:, l * P:(l + 1) * P]
            for half, tg in enumerate(pair):
                m = tg in MASKED_TILES[l]
                nc.tensor.matmul(
                    ps[:, half * TW:(half + 1) * TW],
                    lhsT=lhsT, rhs=xcol(tg), start=True, stop=not m)
                if m:
                    nc.tensor.matmul(
                        ps[:, half * TW:(half + 1) * TW],
                        lhsT=oha[:, l * P:(l + 1) * P],
                        rhs=ohc[:, OH_SLOT[tg] * TW:(OH_SLOT[tg] + 1) * TW],
                        start=False, stop=True)

        for l in range(NBLK):
            containers, exact, soft, order = _block_plan(l)
            for ci in order:
                ps = ppool.tile([P, SC], F32, tag="ps")
                mm_container(ps, l, containers[ci])
                if ci == 0:
                    # hardest-positive softmax over the masked container
                    sm = scpool.tile([P, SC], F32, tag="sm")
                    nc.scalar.activation(
                        sm[:], ps[:], mybir.ActivationFunctionType.Exp,
                        bias=bmx[:, l:l + 1], scale=TSOFT,
                        accum_out=msk[:, l:l + 1])
                if ci in exact:
                    k = exact.index(ci)
                    nc.vector.tensor_reduce(
                        acc_e[:, 5 * l + k:5 * l + k + 1], ps[:],
                        op=mybir.AluOpType.min, axis=mybir.AxisListType.X)
                else:
                    j = soft.index(ci)
                    sb = scpool.tile([P, SC], F32, tag="sb")
                    nc.scalar.activation(
                        sb[:], ps[:], mybir.ActivationFunctionType.Exp,
                        bias=bmn[:, l:l + 1], scale=-TSOFT,
                        accum_out=ssm[:, 4 * l + j:4 * l + j + 1])

        outt = fpool.tile([P, OUT_W], F32, tag="outt")
        nc.vector.tensor_copy(outt[:, 0:5 * NBLK], acc_e[:])
        nc.vector.tensor_copy(outt[:, 5 * NBLK:9 * NBLK], ssm[:])
        nc.vector.tensor_copy(outt[:, 9 * NBLK:10 * NBLK], msk[:])
        nc.sync.dma_start(out_d.ap()[:, :], outt[:])
    return nc


def _reference_fallback(embeddings, labels):
    x = embeddings / np.maximum(
        np.sqrt((embeddings * embeddings).sum(1, keepdims=True)), 1e-12)
    sq = (x * x).sum(1)
    d2 = sq[:, None] + sq[None, :] - 2.0 * (x @ x.T)
    dist = np.sqrt(np.maximum(d2, 0.0))
    same = labels[:, None] == labels[None, :]
    eye = np.eye(len(labels), dtype=bool)
    pos, neg = same & ~eye, ~same
    d_ap = np.where(pos, dist, -np.inf).max(1)
    d_an = np.where(neg, dist, np.inf).min(1)
    valid = pos.any(1) & neg.any(1)
    per = np.maximum(d_ap - d_an + MARGIN, 0.0)
    per = np.where(valid, per, 0.0)
    nz = (per > 0).sum()
    return np.float32(per.sum() / max(nz, 1)) if nz > 0 else np.float32(0.0)


def kernel(embeddings: np.ndarray, labels: np.ndarray) -> np.ndarray:
    global LAST_RESULTS
    emb = np.asarray(embeddings, dtype=np.float32)
    lab = np.asarray(labels).reshape(-1)

    counts = np.bincount(lab.astype(np.int64) - lab.min())
    if emb.shape != (N, D) or counts.max() > 256 or len(np.unique(lab)) < 2:
        return np.array(_reference_fallback(emb, lab), dtype=np.float32)

    norms = np.sqrt((emb * emb).sum(1, keepdims=True, dtype=np.float32))
    xn = emb / np.maximum(norms, np.float32(1e-12))
    s = (xn * xn).sum(1, dtype=np.float32)

    perm = np.argsort(lab, kind="stable")
    xs = xn[perm]
    ls = lab[perm]
    ss = s[perm]

    uniq = np.unique(ls)
    code = np.searchsorted(uniq, ls).astype(np.int64)
    assert len(uniq) <= 64

    # fp8 operand planes (device sees these exact values)
    xs8 = xs.astype(E4M3)
    xs8f = xs8.astype(np.float32)
    xa8 = (-2.0 * xs).astype(E4M3)
    xa8f = xa8.astype(np.float32)

    rng = np.random.default_rng(7)
    # hardest-positive softmax reference: sampled same-label max (lifted)
    r_mx = np.empty(N, dtype=np.float32)
    for g in range(len(uniq)):
        idx = np.where(code == g)[0]
        samp = rng.choice(idx, size=min(16, len(idx)), replace=False)
        gm = xs8f[idx] @ xs8f[samp].T
        r_mx[idx] = (-2.0 * gm).max(1) + np.float32(BIG)

    # per (core, block) soft columns from the static plan
    soft_cols_cl = {}
    for l in range(NBLK):
        containers, exact, soft, _ = _block_plan(l)
        cols = []
        for ci in soft:
            for tg in containers[ci]:
                cols.extend(range(tg * TW, (tg + 1) * TW))
        soft_cols_cl[l] = np.array(cols)

    # softmin reference: sampled min over the block's soft columns
    r_mn = np.empty(N, dtype=np.float32)
    for c in range(NCORES):
        lo = c * M
        rot = np.roll(np.arange(N), -lo)
        for l in range(NBLK):
            rows = np.arange(lo + l * P, lo + (l + 1) * P)
            sc = soft_cols_cl[l]
            samp = rng.choice(sc, size=96, replace=False)
            srows = rot[samp]
            v = -2.0 * (xs8f[rows] @ xs8f[srows].T)
            v += BIG * (ls[rows][:, None] == ls[srows][None, :])
            r_mn[rows] = v.min(1)

    _install_wait_split_patch()
    _ensure_ntff_hook()
    nc = _build_nc()

    in_maps = []
    for c in range(NCORES):
        lo = c * M
        rot = np.roll(np.arange(N), -lo)
        xt = np.ascontiguousarray(xs8[rot].T)
        xm2a = np.ascontiguousarray(xa8[lo:lo + M].T)
        slab = np.concatenate([rot[N - TW:], rot[:3 * TW]])
        ohc = (code[slab][None, :] == np.arange(64)[:, None]).astype(E4M3)
        oha = (BIG * (code[lo:lo + M][None, :] == np.arange(64)[:, None])).astype(E4M3)
        bmn = np.ascontiguousarray(
            (TSOFT * r_mn[lo:lo + M]).reshape(NBLK, P).T.astype(np.float32))
        bmx = np.ascontiguousarray(
            (-TSOFT * r_mx[lo:lo + M]).reshape(NBLK, P).T.astype(np.float32))
        in_maps.append({"xt": xt, "xm2a": xm2a, "ohc": ohc, "oha": oha,
                        "bmn": bmn, "bmx": bmx})

    res = run_bass_kernel_spmd(nc, in_maps, core_ids=list(range(NCORES)))
    LAST_RESULTS = res

    d_ap_all = np.empty(N, dtype=np.float64)
    d_an_all = np.empty(N, dtype=np.float64)
    bad = []
    for c in range(NCORES):
        o = np.asarray(res.results[c]["per_out"], dtype=np.float64)
        lo = c * M
        for l in range(NBLK):
            rows = np.arange(lo + l * P, lo + (l + 1) * P)
            e = BLOCK_E[l]
            m_exact = o[:, 5 * l:5 * l + e].min(1)
            ssum = o[:, 5 * NBLK + 4 * l:5 * NBLK + 4 * l + (8 - e)].sum(1)
            msum = o[:, 9 * NBLK + l]
            ok = np.isfinite(ssum) & np.isfinite(msum) & (msum > 0)
            bad.extend(rows[~ok])
            mn_soft = r_mn[rows] - np.log(np.maximum(ssum, 1e-30)) / TSOFT
            mn = np.minimum(m_exact, mn_soft)
            mx = r_mx[rows] + np.log(np.maximum(msum, 1e-30)) / TSOFT
            s_i = ss[rows]
            d_an_all[rows] = np.sqrt(np.maximum(s_i + 1.0 + mn, 0.0))
            d_ap_all[rows] = np.sqrt(np.maximum(s_i + 1.0 + mx - BIG, 0.0))

    if bad:
        # overflowed/degenerate anchors: recompute exactly on host (rare)
        for i in bad:
            g = xs8f @ xs8f[i]
            d2 = ss + ss[i] - 2.0 * g
            d = np.sqrt(np.maximum(d2, 0.0))
            samel = ls == ls[i]
            posm = samel.copy()
            posm[i] = False
            d_ap_all[i] = d[posm].max() if posm.any() else 0.0
            d_an_all[i] = d[~samel].min()

    per = np.maximum(d_ap_all - d_an_all + MARGIN, 0.0)
    nz = int((per > 0).sum())
    if nz == 0:
        return np.array(0.0, dtype=np.float32)
    return np.array(np.float32(per.sum() / nz), dtype=np.float32)


if __name__ == "__main__":
    from concourse import bass_utils
    import tempfile
    _install_wait_split_patch()
    nc = _build_nc()
    td = tempfile.mkdtemp(prefix="tripletk_")
    print(bass_utils.compile_bass_kernel(nc, td))
